# revision 28
# baseline (speedup 1.0000x reference)
"""Bass/Tile TRN2 kernel for a batched self-attention layer.

Reference computation (per batch b, N = 64*64 = 4096 tokens, C = 256, Dp = 32):
    f = input_h @ f_w          [N, Dp]
    g = x @ g_w                [N, Dp]
    s = g @ f.T                [N, N]
    beta = softmax(s, -1)
    o = beta @ input_h         [N, C]
    out = concat([o, x], -1)   [N, 2C]

Sharding: 8 cores = (batch b, query-half) pairs. Each core handles 2048 query
rows of one batch with the full 4096-key attention for that batch.

v5 design notes:
  * All layout work (transposes, fp16/bf16 casts, ones-column append) happens
    on the HOST; the device runs only matmuls + exp + normalize.
  * Attention in TRANSPOSED layout per 512-query block, two chunk pairs per
    pipeline step: sT[key,q] chunk pairs via two concurrent K=32 row-tiled
    matmuls (tile_position) into double-buffered 2-bank PSUM tiles; exp
    (fp32-range, no max subtraction) straight from PSUM into bf16 SBUF; PV
    accumulates exp_chunk.T @ hR_chunk into 4 fp32 PSUM accumulators over the
    32 key chunks, a ones column yielding the softmax denominator for free.
  * Input DMAs use large per-partition descriptors (1-4KB) — fine-grained
    span splitting measurably starves the DMA engines. Order: xT (gates the
    gT projections), hT halves (gate fT), then hR gated behind hT via tiny
    GpSimd touch ops (real WAW dependencies; GpSimd is idle so no FIFO
    head-of-line blocking).
  * PE warm-up matmuls + a dummy exp run during the initial DMA so the HAM
    clock gate is at 2.4 GHz and the ACT exp table is loaded when real work
    starts. Deferred normalization (DVE reciprocal + scalar-mul) of each
    block hides in the next block's pipeline ramp.
"""

import numpy as np
import ml_dtypes

import concourse.bass as bass
import concourse.tile as tile
from concourse import bacc
from concourse import mybir
from concourse.bass_utils import run_bass_kernel_spmd

F32 = mybir.dt.float32
F16 = mybir.dt.float16
BF16 = mybir.dt.bfloat16

B, W, C, D = 4, 64, 256, 32
N = W * W                 # 4096 tokens (keys) per batch
NCORES = 8
SHARDS_PER_BATCH = NCORES // B   # 2
NQ = N // SHARDS_PER_BATCH       # 2048 query rows per core
KC = 128                         # key chunk (PE partition dim)
NKC = N // KC                    # 32 key chunks
QBLK = 512                       # query block (moving free dim)
NQB = NQ // QBLK                 # 4 query blocks per core
QSUB = 128                       # query sub-tile (PV stationary M)
NQSUB = QBLK // QSUB             # 4
NP = NKC // 2                    # 16 chunk pairs per query block
NSTEP = NP // 2                  # 8 two-pair pipeline steps
NWARM = 10                       # PE warm-up matmuls during input DMA
Exp = mybir.ActivationFunctionType.Exp


def _build() -> bass.Bass:
    nc = bacc.Bacc("TRN2", target_bir_lowering=False)

    xT = nc.declare_dram_parameter("xT", [C, NQ], F16, isOutput=False)
    hT = nc.declare_dram_parameter("hT", [C, N], F16, isOutput=False)
    hR = nc.declare_dram_parameter("hR", [N, C + 2], BF16, isOutput=False)
    fwg = nc.declare_dram_parameter("fwg", [128, 4 * D], F16, isOutput=False)
    o = nc.declare_dram_parameter("o", [NQ, C], F32, isOutput=True)

    with tile.TileContext(nc) as tc:
        with (
            tc.tile_pool(name="const", bufs=1) as const_pool,
            tc.tile_pool(name="hr", bufs=1) as hr_pool,
            tc.tile_pool(name="inp", bufs=1) as inp_pool,
            tc.tile_pool(name="proj", bufs=1) as proj_pool,
            tc.tile_pool(name="esb", bufs=4) as e_pool,
            tc.tile_pool(name="osb", bufs=4) as out_pool,
            tc.tile_pool(name="rsb", bufs=4) as r_pool,
            tc.tile_pool(name="ops", bufs=1, space="PSUM") as o_pool,
        ):
            zbias = const_pool.tile([128, 1], F32)
            nc.vector.memset(zbias[:, :], 0.0)
            warm = const_pool.tile([128, C + 2], F16)
            nc.vector.memset(warm[:, :], 0.0)
            # Dummy activation pulls the ~2.7us exp table load off the
            # critical path (runs during the input DMA).
            actwarm = const_pool.tile([128, 1], F32)
            nc.scalar.activation(actwarm[:, :], zbias[:, :], Exp, bias=zbias[:, :])

            fwg_sb = const_pool.tile([128, 4 * D], F16)
            nc.sync.dma_start(out=fwg_sb[:, :], in_=fwg[:, :])

            # PE warm-up: junk matmuls on zeroed SBUF while DMA lands; they
            # target the o0 accumulator bank, which attention reuses later.
            wps = o_pool.tile([128, C + 2], F32, tag="o0", name="warm")
            for wi in range(NWARM):
                nc.tensor.matmul(wps[:, :], warm[:, 0:128], warm[:, :], start=True, stop=True)

            xT_sb = [inp_pool.tile([128, NQ], F16, tag=f"xT{cc}", name=f"xT{cc}") for cc in range(2)]
            hT_sb = [inp_pool.tile([128, N], F16, tag=f"hT{cc}", name=f"hT{cc}") for cc in range(2)]
            hr_blk = [
                hr_pool.tile([128, 4, C + 2], BF16, tag=f"hr{p}", name=f"hr{p}")
                for p in range(NKC // 4)
            ]

            # xT and hT as whole-tile transfers: 4-8KB per-partition
            # descriptors measurably beat smaller pieces on aggregate DMA
            # throughput (each dma_start's descriptors already spread across
            # the 16 queues). hR is gated behind them (below).
            for cc in range(2):
                nc.sync.dma_start(out=xT_sb[cc][:, :], in_=xT[cc * 128:(cc + 1) * 128, :])
            # tile_wait_until = modeled-time annotation only (no runtime
            # wait): tells the static scheduler the hT halves land late (HBM
            # is shared by all 8 cores during the ramp), so early attention
            # steps get queued BEFORE the late fT projections instead of
            # stalling behind them.
            for h in range(2):
                with tc.tile_wait_until(0.006 + 0.005 * h):
                    for cc in range(2):
                        nc.sync.dma_start(
                            out=hT_sb[cc][:, h * 2048:(h + 1) * 2048],
                            in_=hT[cc * 128:(cc + 1) * 128, h * 2048:(h + 1) * 2048],
                        )

            def dma_gated_inputs():
                # GpSimd touches (idle engine, so no FIFO head-of-line
                # blocking) sequence the hR transfers after the hT spans.
                gate = hT_sb[0][0:1, N - 1:N]
                for p in range(NKC // 4):
                    nc.gpsimd.tensor_copy(hr_blk[p][0:1, 0:1, 0:1], gate)
                    # Host pre-permuted: chunk k = 4*blk + j holds keys 128k..128k+127.
                    nc.sync.dma_start(
                        out=hr_blk[p][:, :, :],
                        in_=hR[p * 512:(p + 1) * 512, :].rearrange("(p j) c -> p j c", p=128),
                    )

            # fT/gT in fp16, chunk PAIRS interleaved across PE row groups
            # (rows 32i hold chunk 2g+i) so two K=32 QK matmuls run
            # concurrently via tile_position row tiling. gT rows 32:64
            # duplicate rows 0:32 (one copy per row group).
            fT2_sb = proj_pool.tile([2 * D, NP, 128], F16)
            gT2_sb = proj_pool.tile([2 * D, NQB, QBLK], F16)

            # The projections borrow the four o-accumulator PSUM banks (in
            # 256-column half-span pieces that fit the 258-column tiles)
            # instead of a scoped pool: a scoped pool's bank handoff to the
            # attention s-pool would serialize ALL of proj before ANY
            # attention work. Rotating o-tags keeps four pieces in flight.
            _ocnt = [0]

            def otile(name):
                _ocnt[0] += 1
                return o_pool.tile([128, C + 2], F32, tag=f"o{_ocnt[0] % 4}", name=name)

            def proj_g(qb):
                for hq in range(2):
                    st = otile(f"gp{qb}_{hq}")
                    q0 = qb * QBLK + hq * 256
                    for i in range(2):
                        for cc in range(2):
                            nc.tensor.matmul(
                                st[32 * i:32 * (i + 1), 0:256],
                                fwg_sb[:, cc * 2 * D + D:cc * 2 * D + 2 * D],
                                xT_sb[cc][:, q0:q0 + 256],
                                start=(cc == 0),
                                stop=(cc == 1),
                                tile_position=(0, 32 * i),
                            )
                    nc.vector.tensor_copy(
                        gT2_sb[:, qb, hq * 256:(hq + 1) * 256], st[0:2 * D, 0:256]
                    )

            def proj_f(s):
                # half h holds chunks 4s+2h (rows 0:32) and 4s+2h+1 (32:64).
                for h in range(2):
                    st = otile(f"fp{s}_{h}")
                    k0 = s * 512 + h * 256
                    for i in range(2):
                        for cc in range(2):
                            nc.tensor.matmul(
                                st[32 * i:32 * (i + 1), 0:256],
                                fwg_sb[:, cc * 2 * D:cc * 2 * D + D],
                                hT_sb[cc][:, k0:k0 + 256],
                                start=(cc == 0),
                                stop=(cc == 1),
                                tile_position=(0, 32 * i),
                            )
                    nc.vector.tensor_copy(fT2_sb[0:32, 2 * s + h, :], st[0:32, 0:128])
                    nc.vector.tensor_copy(fT2_sb[32:64, 2 * s + h, :], st[32:64, 128:256])

            for qb in range(NQB):
                proj_g(qb)
            for s in range(8):
                proj_f(s)
            dma_gated_inputs()

            def pv(o_ps, e_ap, k):
                for i in range(NQSUB):
                    nc.tensor.matmul(
                        o_ps[i][:, :],
                        e_ap[:, i * 128:(i + 1) * 128],
                        hr_blk[k // 4][:, k % 4, :],
                        start=(k == 0),
                        stop=(k == NKC - 1),
                    )

            def norm_out(qb, o_ps):
                for i in range(NQSUB):
                    rec = r_pool.tile([128, 1], F32, tag="rec", name=f"rec{qb}_{i}")
                    nc.vector.reciprocal(rec[:, :], o_ps[i][:, C:C + 1])
                    out_sb = out_pool.tile([128, C], F32, tag="ob", name=f"ob{qb}_{i}")
                    nc.vector.tensor_scalar_mul(out_sb[:, :], o_ps[i][:, 0:C], rec[:, :])
                    r0 = qb * QBLK + i * 128
                    nc.sync.dma_start(out=o[r0:r0 + 128, :], in_=out_sb[:, :])

            # --- attention: steps of two chunk pairs ---
            # step pipeline: [QK pair, QK pair](t+1) -> [exp, exp](t) -> [16x PV](t)
            with tc.tile_pool(name="sps", bufs=2, space="PSUM") as s_pool:
                pending_norm = None
                for qb in range(NQB):
                    o_ps = [
                        o_pool.tile([128, C + 2], F32, tag=f"o{i}", name=f"ops{qb}_{i}")
                        for i in range(NQSUB)
                    ]

                    def qk_pair(g, qb=qb):
                        s_ps = s_pool.tile([128, 2, QBLK], F32, tag="s", name=f"sps{qb}_{g}")
                        for half in range(2):
                            nc.tensor.matmul(
                                s_ps[:, half, :],
                                fT2_sb[32 * half:32 * (half + 1), g, :],
                                gT2_sb[32 * half:32 * (half + 1), qb, :],
                                start=True,
                                stop=True,
                                tile_position=(32 * half, 0),
                            )
                        return s_ps

                    prev = [(0, qk_pair(0)), (1, qk_pair(1))]
                    if pending_norm is not None:
                        norm_out(*pending_norm)
                        pending_norm = None
                    for t in range(NSTEP):
                        nxt = None
                        if t + 1 < NSTEP:
                            nxt = [(2 * t + 2, qk_pair(2 * t + 2)), (2 * t + 3, qk_pair(2 * t + 3))]
                        es = []
                        for gp, s_ps in prev:
                            e_sb = e_pool.tile([128, 2, QBLK], BF16, tag="e", name=f"e{qb}_{gp}")
                            nc.scalar.activation(e_sb[:, :, :], s_ps[:, :, :], Exp, bias=zbias[:, :])
                            es.append((gp, e_sb))
                        for ge, e in es:
                            for half in range(2):
                                pv(o_ps, e[:, half, :], 2 * ge + half)
                        prev = nxt
                    pending_norm = (qb, o_ps)
                if pending_norm is not None:
                    norm_out(*pending_norm)

    nc.finalize()
    return nc


_CACHE: dict = {}


def _get_nc() -> bass.Bass:
    if "nc" not in _CACHE:
        _CACHE["nc"] = _build()
    return _CACHE["nc"]


def _prep_batch(hf_b):
    """Per-batch host prep shared by both query-half cores."""
    hT = np.ascontiguousarray(hf_b.T.astype(np.float16))              # [C, N]
    aug = np.empty((N, C + 2), dtype=ml_dtypes.bfloat16)
    aug[:, 0:C] = hf_b.astype(ml_dtypes.bfloat16)
    aug[:, C] = 1.0
    aug[:, C + 1] = 0.0
    # chunk k = 4*blk + j holds keys 128k..128k+127: [blk, j, p, c] -> [blk, p, j, c]
    hR = np.ascontiguousarray(
        aug.reshape(NKC // 4, 4, 128, C + 2).transpose(0, 2, 1, 3).reshape(N, C + 2)
    )
    return hT, hR


def _shard(x, input_h, f_w, g_w):
    xf = np.asarray(x, dtype=np.float32).reshape(B, N, C)
    hf = np.asarray(input_h, dtype=np.float32).reshape(B, N, C)
    fwf = np.asarray(f_w, dtype=np.float32).reshape(C, D)
    gwf = np.asarray(g_w, dtype=np.float32).reshape(C, D)
    fwg = np.empty((128, 4 * D), dtype=np.float16)
    for cc in range(2):
        fwg[:, cc * 2 * D:cc * 2 * D + D] = fwf[cc * 128:(cc + 1) * 128, :]
        fwg[:, cc * 2 * D + D:cc * 2 * D + 2 * D] = gwf[cc * 128:(cc + 1) * 128, :]
    per_batch = [_prep_batch(hf[b]) for b in range(B)]
    in_maps = []
    for c in range(NCORES):
        b, half = divmod(c, SHARDS_PER_BATCH)
        hTb, hRb = per_batch[b]
        xTc = np.ascontiguousarray(
            xf[b, half * NQ:(half + 1) * NQ].T.astype(np.float16)
        )
        in_maps.append({"xT": xTc, "hT": hTb, "hR": hRb, "fwg": fwg})
    return in_maps


def _gather(results, x):
    of = np.empty((B, N, C), np.float32)
    for c in range(NCORES):
        b, half = divmod(c, SHARDS_PER_BATCH)
        of[b, half * NQ:(half + 1) * NQ] = results[c]["o"]
    o4 = of.reshape(B, W, W, C)
    x4 = np.asarray(x, dtype=np.float32).reshape(B, W, W, C)
    return np.concatenate([o4, x4], axis=-1)


def run(inputs: dict, trace: bool = False):
    """Run the kernel; returns (full_output, BassKernelResults)."""
    in_maps = _shard(**inputs)
    res = run_bass_kernel_spmd(_get_nc(), in_maps, list(range(NCORES)), trace=trace)
    out = _gather(res.results, inputs["x"])
    return out, res


def kernel(**inputs) -> np.ndarray:
    out, _ = run(inputs, trace=False)
    return out
